# revision 45
# baseline (speedup 1.0000x reference)
"""BinaryMeanpass3d Trainium2 kernel.

Math: the reference's mean-field iteration m <- damped sigmoid(energy(m)) is a strong
contraction with a unique fixed point (r in [0, 0.25) keeps it contractive); its output
is the fully converged fixed point to f32 precision (the reference's own convergence
check passes with diff ~6e-8 after one outer block). We therefore compute the same
fixed point directly with undamped sweeps in q-space (q = 2m - 1):
    q <- tanh(0.5 * (d + sum_axis [ r * shift+(q) + shift-(r * q) ]))
then emit energy(q).

Distribution: volume (96,128,128) sharded along D over 8 cores (12 slices each).
No inter-core communication: each core loads its 12 slices plus a K-deep halo and
runs K sweeps on a window whose valid region shrinks by one slice per side per sweep
(communication-free temporal blocking). Zero-padded ghost slices with r=0 reproduce
the reference's one-sided boundary handling exactly, and make all 8 cores run an
identical SPMD program.

On-chip layout: SBUF tensors [partitions = H = 128, free = window_slices * W].
Per sweep chunk: VectorE+GpSimd compute 6 elementwise products (free-dim shifts are
AP offsets; the partition-dim shift of the static ry is precomputed on host as rys),
TensorE accumulates the 7 stencil terms into PSUM via identity / partition-shift
matmuls, ScalarE applies tanh(0.5*e) from PSUM back to SBUF.

Precision phases: the first K-K_LATE sweeps use bf16 q/r/products (DVE 2x mode, PE
full rate); the last K_LATE sweeps use f32 q / f32 r with float32r-rounded products
(full-rate PE, ~1e-5 rounding), which contracts the bf16-phase error away; the final
energy pass re-adds unrounded f32 d. The d-term matmul is float32r in all phases.
"""

import numpy as np
import ml_dtypes

import concourse.bacc as bacc
import concourse.mybir as mybir
from concourse.tile import TileContext
from concourse.bass_utils import run_bass_kernel_spmd

D, H, W = 96, 128, 128
NCORES = 8
DLOC = D // NCORES          # 12 owned slices per core
K = 7                       # sweeps (windowing err ~4e-5, ~the fp32r floor)
K_LATE = 2                  # trailing sweeps at f32/fp32r precision
KE = K - K_LATE             # leading bf16 sweeps
PAD = 1                     # zero pad slices at each window end (for shifted reads)
WTOT = DLOC + 2 * K + 2 * PAD   # 30 window slices per core
FD = WTOT * W               # free dim of the main SBUF tensors
BANK = 512                  # PSUM bank free-dim (matmul max moving free dim)
CSL_B = 8                   # slices per chunk, bf16 phase (1024 elements)
CSL_R = 4                   # slices per chunk, fp32r phase (512 elements)

FP32 = mybir.dt.float32
FP32R = mybir.dt.float32r
BF16 = mybir.dt.bfloat16

last_results = None         # BassKernelResults of the most recent run (for profiling)


def _emit_chunk(nc, ctxk, sl0, nsl, s, dest):
    """One chunk of sweep s (or the final pass when s == K): products -> PSUM -> out."""
    bf = s < KE
    c0 = sl0 * W
    cw = nsl * W
    v, g = nc.vector, nc.gpsimd

    if bf:
        q_in = ctxk["qb"][s % 2]
        rx_s, ry_s, rys_s, rz_s = ctxk["rb"]
        cI, cSu, cSd = ctxk["cmb"]
        prods = ctxk["pb"][ctxk["gchunk"] % 2]
    else:
        # the first late sweep still reads the bf16 q written by sweep KE-1
        q_in = ctxk["qb"][s % 2] if s == KE else ctxk["qf"][s % 2]
        rx_s, ry_s, rys_s, rz_s = ctxk["rf"]
        cI, cSu, cSd = ctxk["cmr"]
        prods = ctxk["pr"][ctxk["gchunk"] % 2]
    ctxk["gchunk"] += 1
    p2, p3, p6, p7, p4, p5 = prods

    # GpSimd product first (primes the Pool queue), then DVE products in PE
    # consumption order so PE trails the producers closely
    # P6[i] = rz[i-1]*q[i-1]       (e[w] += rz[w-1] q[w-1]; rz col127=0 kills wraps)
    g.tensor_mul(p6[:, :cw], q_in[:, c0 - 1:c0 - 1 + cw], rz_s[:, c0 - 1:c0 - 1 + cw])
    # P2[i] = rx[i-1sl]*q[i-1sl]   (e[d] += rx[d-1] q[d-1])
    v.tensor_mul(p2[:, :cw], q_in[:, c0 - W:c0 - W + cw], rx_s[:, c0 - W:c0 - W + cw])
    # P3[i] = rx[i]*q[i+1sl]       (e[d] += rx[d] q[d+1])
    v.tensor_mul(p3[:, :cw], q_in[:, c0 + W:c0 + W + cw], rx_s[:, c0:c0 + cw])
    # P7[i] = rz[i]*q[i+1]         (e[w] += rz[w] q[w+1])
    e7 = v if bf else g  # bf16 DVE muls are 2x -- keep only P6 on the Q7s there
    e7.tensor_mul(p7[:, :cw], q_in[:, c0 + 1:c0 + 1 + cw], rz_s[:, c0:c0 + cw])
    # P4 = rys*q, rys[h] = ry[h-1]; via S_up: e[h] += ry[h] q[h+1]
    e4 = g if (not bf and ctxk["gchunk"] % 3 == 0) else v  # late-phase load balance
    e4.tensor_mul(p4[:, :cw], q_in[:, c0:c0 + cw], rys_s[:, c0:c0 + cw])
    # P5 = ry*q; via S_dn: e[h] += ry[h-1] q[h-1]
    v.tensor_mul(p5[:, :cw], q_in[:, c0:c0 + cw], ry_s[:, c0:c0 + cw])

    mm = nc.tensor.matmul
    with_d = dest[0] == "tanh"
    # group by lhsT across banks to minimize weight switches; accumulation
    # groups are per-bank (start on first write, stop on last); bank-sized
    # PSUM tiles (8-deep rotation) for finer cross-chunk overlap
    groups = []
    if with_d:
        # d term is float32r in every phase (full-rate, ~1e-5 rounding)
        groups.append((ctxk["cmr"][0], [("d", None)]))
    else:
        # final pass: unrounded d via a plain-f32 matmul (1/4-rate PE is idle
        # here) so the output keeps full f32 d precision
        groups.append((ctxk["cI32"], [("df", None)]))
    groups.append((cI, [(None, p) for p in (p2, p3, p6, p7)]))
    groups.append((cSu, [(None, p4)]))
    groups.append((cSd, [(None, p5)]))
    banks = [(j0, min(BANK, cw - j0)) for j0 in range(0, cw, BANK)]
    btiles = {j0: ctxk["psum"].tile([H, bw], FP32, name="ps") for j0, bw in banks}
    d_stage = None if with_d else dest[4]
    for gi, (wt, rhss) in enumerate(groups):
        for ri, (tag, p) in enumerate(rhss):
            for j0, bw in banks:
                if tag == "d":
                    rhs = ctxk["d_r"][:, c0 + j0:c0 + j0 + bw]
                elif tag == "df":
                    rhs = d_stage[:, j0:j0 + bw]
                else:
                    rhs = p[:, j0:j0 + bw]
                mm(btiles[j0][:, :bw], wt, rhs,
                   start=(gi == 0 and ri == 0),
                   stop=(gi == len(groups) - 1))

    if dest[0] == "tanh":
        for j0, bw in banks:
            nc.scalar.activation(dest[1][:, c0 + j0:c0 + j0 + bw], btiles[j0][:, :bw],
                                 mybir.ActivationFunctionType.Tanh, scale=0.5)
    else:
        # final energy: PSUM -> SBUF on the idle ACT, then DMA out
        _, out_ap, oc, stage, _ = dest
        for j0, bw in banks:
            nc.scalar.copy(out=stage[:, j0:j0 + bw], in_=btiles[j0][:, :bw])
        nc.sync.dma_start(out=out_ap[:, oc:oc + cw], in_=stage[:, :cw])


def _build():
    nc = bacc.Bacc("TRN2", debug=False, num_devices=NCORES, enable_asserts=False)

    d_d = nc.dram_tensor("d", [H, FD], FP32, kind="ExternalInput")
    rx_d = nc.dram_tensor("rx", [H, FD], FP32, kind="ExternalInput")
    ry_d = nc.dram_tensor("ry", [H, FD], FP32, kind="ExternalInput")
    rys_d = nc.dram_tensor("rys", [H, FD], FP32, kind="ExternalInput")
    rz_d = nc.dram_tensor("rz", [H, FD], FP32, kind="ExternalInput")
    rb_d = nc.dram_tensor("rb", [H, 4 * FD + 3 * 128], BF16, kind="ExternalInput")
    cm_d = nc.dram_tensor("cm", [128, 3 * 128], FP32, kind="ExternalInput")
    out_d = nc.dram_tensor("out", [H, DLOC * W], FP32, kind="ExternalOutput")

    with TileContext(nc) as tc:
        with tc.tile_pool(name="main", bufs=1) as pool, \
             tc.tile_pool(name="psum", bufs=8, space="PSUM") as psum_pool:
            stf = pool.tile([H, 4 * FD], FP32)            # f32 statics (late phase)
            stb = pool.tile([H, 4 * FD + 3 * 128], BF16)  # bf16 statics + matrices
            d_r = pool.tile([H, FD], FP32R)
            cm_r = pool.tile([128, 3 * 128], FP32R)
            cI32 = pool.tile([128, 128], FP32)  # f32 identity for the final d-term
            qA = pool.tile([H, FD], FP32)
            qB = pool.tile([H, FD], FP32)

            ctxk = {
                "rf": tuple(stf[:, i * FD:(i + 1) * FD] for i in range(4)),
                "rb": tuple(stb[:, i * FD:(i + 1) * FD] for i in range(4)),
                "cmb": tuple(stb[:, 4 * FD + i * 128:4 * FD + (i + 1) * 128]
                             for i in range(3)),
                "cmr": tuple(cm_r[:, i * 128:(i + 1) * 128] for i in range(3)),
                "d_r": d_r,
                "cI32": cI32[:, :],
                "qf": (qA, qB),
                "qb": (qA.bitcast(BF16)[:, 0:FD], qB.bitcast(BF16)[:, 0:FD]),
                "pb": [[pool.tile([H, CSL_B * W], BF16, name=f"pb{t}_{si}")
                        for t in range(6)] for si in range(2)],
                "pr": [[pool.tile([H, CSL_R * W], FP32R, name=f"pr{t}_{si}")
                        for t in range(6)] for si in range(2)],
                "psum": psum_pool,
                "gchunk": 0,
            }

            # --- loads: bf16 pack + d first (they gate the early sweeps); f32
            # statics later (only needed from sweep KE, overlap with compute)
            nc.sync.dma_start(out=qA[:, 0:3 * 128], in_=cm_d.ap())
            nc.scalar.copy(out=cm_r[:, :], in_=qA[:, 0:3 * 128])
            nc.sync.dma_start(out=cI32[:, :], in_=cm_d.ap()[:, 0:128])
            NT = 3
            cuts = [(FD * i // NT) // W * W for i in range(NT)] + [FD]
            # per-field thirds ordered by first consumer: d (q0/tanh), then
            # rx (DVE), rz (GpSimd), rys/ry; the tiny matrices ride first
            nc.sync.dma_start(out=stb[:, 4 * FD:], in_=rb_d.ap()[:, 4 * FD:])
            for i in range(NT):
                a, b = cuts[i], cuts[i + 1]
                nc.sync.dma_start(out=qB[:, a:b], in_=d_d.ap()[:, a:b])
                for f in (0, 3, 2, 1):  # rx, rz, rys, ry within the pack
                    nc.sync.dma_start(out=stb[:, f * FD + a:f * FD + b],
                                      in_=rb_d.ap()[:, f * FD + a:f * FD + b])
            for i in range(NT):
                a, b = cuts[i], cuts[i + 1]
                # q0 = tanh(0.5*d) in bf16; d_r = fp32r round of d (on idle DVE)
                nc.scalar.activation(ctxk["qb"][0][:, a:b], qB[:, a:b],
                                     mybir.ActivationFunctionType.Tanh, scale=0.5)
                nc.vector.tensor_copy(out=d_r[:, a:b], in_=qB[:, a:b])
            for i, dram in enumerate((rx_d, ry_d, rys_d, rz_d)):
                nc.sync.dma_start(out=stf[:, i * FD:(i + 1) * FD], in_=dram.ap())

            # --- sweeps
            for s in range(K):
                lo, hi = 1 + s, WTOT - 1 - s
                csl = CSL_B if s < KE else CSL_R
                q_out = ctxk["qb"][(s + 1) % 2] if s < KE else ctxk["qf"][(s + 1) % 2]
                for sl0 in range(lo, hi, csl):
                    nsl = min(csl, hi - sl0)
                    _emit_chunk(nc, ctxk, sl0, nsl, s, ("tanh", q_out))

            # --- final energy on the 12 owned slices from q_fin = qf[K%2].
            # The other q tensor is dead now; reuse it for the unrounded-d
            # reload + out staging.
            assert K_LATE >= 1
            # stage in columns of the dead q tensor that no sweep after ~s=4
            # touches, so the d reload DMA hides under the late sweeps
            dead_q = ctxk["qf"][(K + 1) % 2]
            lo = K + PAD
            out_ap = out_d.ap()
            for sl0 in range(lo, lo + DLOC, CSL_R):
                nsl = min(CSL_R, lo + DLOC - sl0)
                c0 = sl0 * W
                d_stage = dead_q[:, 0:nsl * W]
                stage = dead_q[:, 512:512 + nsl * W]
                nc.sync.dma_start(out=d_stage, in_=d_d.ap()[:, c0:c0 + nsl * W])
                _emit_chunk(nc, ctxk, sl0, nsl, K,
                            ("out", out_ap, (sl0 - lo) * W, stage, d_stage))

    nc.compile()  # bacc register allocation / lowering
    return nc


_nc_cache = None


def kernel(d, rx, ry, rz):
    global _nc_cache, last_results
    dv = np.asarray(d, dtype=np.float32).reshape(D, H, W)
    rxv = np.asarray(rx, dtype=np.float32).reshape(D, H, W).copy()
    ryv = np.asarray(ry, dtype=np.float32).reshape(D, H, W)
    rzv = np.asarray(rz, dtype=np.float32).reshape(D, H, W).copy()
    # entries never read by the reference stencil; zeroing them makes the
    # kernel's wrap-around shifted reads contribute exactly zero
    rxv[D - 1] = 0.0
    rzv[:, :, W - 1] = 0.0
    # partition-shifted copy of ry (rys[h] = ry[h-1]) so the kernel only ever
    # needs partition-aligned elementwise reads
    rysv = np.zeros_like(ryv)
    rysv[:, 1:, :] = ryv[:, :-1, :]

    cm = np.concatenate([
        np.eye(128, dtype=np.float32),          # cI
        np.eye(128, k=-1, dtype=np.float32),    # cSu: out[m] = in[m+1]
        np.eye(128, k=1, dtype=np.float32),     # cSd: out[m] = in[m-1]
    ], axis=1)

    in_maps = []
    for c in range(NCORES):
        lo = c * DLOC - K - PAD
        hi = lo + WTOT
        a, b = max(lo, 0), min(hi, D)
        m = {}
        for name, arr in (("d", dv), ("rx", rxv), ("ry", ryv), ("rys", rysv), ("rz", rzv)):
            win = np.zeros((WTOT, H, W), np.float32)
            win[a - lo:b - lo] = arr[a:b]
            m[name] = np.ascontiguousarray(win.transpose(1, 0, 2).reshape(H, FD))
        m["cm"] = cm
        m["rb"] = np.ascontiguousarray(np.concatenate(
            [m["rx"], m["ry"], m["rys"], m["rz"], cm],
            axis=1).astype(ml_dtypes.bfloat16))
        in_maps.append(m)

    if _nc_cache is None:
        _nc_cache = _build()

    last_results = run_bass_kernel_spmd(_nc_cache, in_maps, core_ids=list(range(NCORES)))

    out = np.zeros((D, H, W), np.float32)
    for c in range(NCORES):
        blk = last_results.results[c]["out"].reshape(H, DLOC, W).transpose(1, 0, 2)
        out[c * DLOC:(c + 1) * DLOC] = blk
    return out.reshape(1, 1, D, H, W)
